# revision 1
# baseline (speedup 1.0000x reference)
"""Trainium2 Bass kernel for nn_GCNConvNet (MFConv GNN, N=100k, E=1.6M).

Strategy (8 NeuronCores, SPMD):
  - Nodes renumbered on host: dealt round-robin per degree-bucket so every
    core owns R rows laid out bucket-contiguously (uniform bucket offsets
    across cores -> one shared program). Pad rows are exactly zero through
    the whole net (biases enter via a host-provided mask row).
  - Edges assigned to the core owning dst. Aggregation h = A @ x runs as:
    dma_gather of src rows from a replicated DRAM table (4 int16 blocks)
    -> one-hot matrices built on DVE (dst_local == iota) -> TensorE
    matmuls accumulate h^T tiles in PSUM -> merged into SBUF.
  - Per-degree-bucket weights applied as dense matmuls over the bucket's
    contiguous column range in the transposed activation layout [d, nodes].
  - fc1/fc2 outputs are computed in both orientations (transposed for the
    next layer's x-side; row-major for the gather table) and the row-major
    tables are AllGathered across the 8 cores.
All FLOPs run on device; the host only does index bookkeeping/sharding.
"""

import math
import os
import sys

sys.path.insert(0, "/opt/trn_rl_repo")

import numpy as np

import concourse.bacc as bacc
import concourse.bass as bass
import concourse.mybir as mybir
import concourse.tile as tile
from concourse import bass_utils
from concourse.library_config import mlp as mlp_lib

F32 = mybir.dt.float32
BF16 = mybir.dt.bfloat16
I16 = mybir.dt.int16

NCORES = 8
P = 128
MAX_DEG = 10
NB = MAX_DEG + 1
SLOPE = 0.01
GATHER_SLOTS = 2048  # target slots per dma_gather call


def _ceil(a, b):
    return (a + b - 1) // b


# ---------------------------------------------------------------------------
# Host-side preprocessing
# ---------------------------------------------------------------------------

class Plan:
    pass


def _preprocess(x, edge_index):
    """Renumber nodes, build per-core slot streams + all metadata."""
    N = x.shape[0]
    E = edge_index.shape[1]
    src = np.asarray(edge_index[0], dtype=np.int64)
    dst = np.asarray(edge_index[1], dtype=np.int64)

    deg = np.bincount(dst, minlength=N).astype(np.int64)
    bucket = np.minimum(deg, MAX_DEG)

    # global order: (bucket, deg) ascending; deal round-robin to cores
    order = np.lexsort((deg, bucket))  # stable by bucket then deg
    core_of = np.empty(N, np.int64)
    rank_of = np.empty(N, np.int64)
    core_of[order] = np.arange(N) % NCORES
    rank_within = np.arange(N) // NCORES  # rank in the dealt sequence

    # per (core, bucket) counts -> uniform padded bucket sizes S_b
    cnt = np.zeros((NCORES, NB), np.int64)
    b_ord = bucket[order]
    c_ord = core_of[order]
    for b in range(NB):
        sel = b_ord == b
        if sel.any():
            cnt[:, b] = np.bincount(c_ord[sel], minlength=NCORES)
    S = cnt.max(axis=0)  # padded per-bucket size, uniform across cores
    off = np.zeros(NB + 1, np.int64)
    off[1:] = np.cumsum(S)
    R = int(math.ceil((off[NB] + 1) / P) * P)

    # local row of each node: bucket offset + rank within (core,bucket)
    # rank within (core,bucket): order of appearance in dealt sequence
    local = np.empty(N, np.int64)
    # nodes in `order` arrive bucket-major; within a bucket, core c's nodes
    # appear in dealt order -> cumulative count per (core,bucket)
    ctr = np.zeros((NCORES, NB), np.int64)
    ob = order
    # vectorized: for nodes sorted by (bucket), the j-th node of (core,bucket)
    # gets local row off[b] + j
    for b in range(NB):
        sel = b_ord == b
        nodes_b = ob[sel]
        cores_b = c_ord[sel]
        # index within core: cumulative count of same core
        idx_in_core = np.zeros(len(nodes_b), np.int64)
        for c in range(NCORES):
            m = cores_b == c
            idx_in_core[m] = np.arange(m.sum())
        local[nodes_b] = off[b] + idx_in_core
    new_global = core_of * R + local  # renumbered global id

    # reverse map per core for unsharding: old node id per local row (-1 pad)
    rows_old = np.full((NCORES, R), -1, np.int64)
    rows_old[core_of, local] = np.arange(N)

    # ---- edge slot streams -------------------------------------------------
    W = R // P  # windows per core
    BLK = 2 * R  # rows per int16 gather block (2 cores per block)
    assert BLK <= 32767, f"block size {BLK} exceeds int16"
    NBLK = 4

    ns = new_global[src]
    nd = new_global[dst]
    ecore = nd // R
    eblock = ns // BLK
    eldst = nd % R
    ewin = eldst // P

    # per (core, block, window) counts -> uniform segment lengths L[b][w]
    key = (eblock * W + ewin) + ecore * (NBLK * W)
    seg_cnt = np.bincount(key, minlength=NCORES * NBLK * W).reshape(
        NCORES, NBLK, W)
    Lseg = seg_cnt.max(axis=0)  # [NBLK, W]
    Lseg = (_ceil_arr(Lseg, P) * P).astype(np.int64)
    M = int(Lseg.sum())

    # slot offsets: block-major, window minor
    seg_off = np.zeros((NBLK, W), np.int64)
    flat = Lseg.reshape(-1)
    seg_off.reshape(-1)[1:] = np.cumsum(flat)[:-1]

    # fill per-core slot arrays
    src_rel = np.zeros((NCORES, M), np.int64)
    dst_loc = np.zeros((NCORES, M), np.int64)
    # zero (pad) row per block: first pad row of core 2b (relative to block)
    zero_rel = np.empty(NBLK, np.int64)
    for b in range(NBLK):
        c = 2 * b
        # find a pad local row on core c (guaranteed: R >= off[NB]+1)
        pad_local = int(off[NB])  # first row past all buckets is padding
        zero_rel[b] = (c % 2) * R + pad_local
    # default src_rel = zero row of the block containing the slot
    for b in range(NBLK):
        s0 = int(seg_off[b, 0])
        s1 = int(seg_off[b, W - 1] + Lseg[b, W - 1])
        src_rel[:, s0:s1] = zero_rel[b]

    eorder = np.lexsort((ns, ewin, eblock, ecore))
    es, eb, ew, ec = ns[eorder], eblock[eorder], ewin[eorder], ecore[eorder]
    el = eldst[eorder]
    # position within segment: running index per (core, block, window)
    seg_pos = np.zeros(E, np.int64)
    k2 = (ec * (NBLK * W) + eb * W + ew)
    # stable sort groups identical keys contiguously -> position = arange - start
    group_starts = np.flatnonzero(np.r_[True, k2[1:] != k2[:-1]])
    lens = np.diff(np.r_[group_starts, E])
    seg_pos = np.arange(E) - np.repeat(group_starts, lens)
    slot = seg_off[eb, ew] + seg_pos
    src_rel[ec, slot] = es % BLK
    dst_loc[ec, slot] = el % P

    # wrap idx arrays: slot i -> [i%16, i//16], replicated to 128 partitions
    idx_wrapped = np.empty((NCORES, P, M // 16), np.int16)
    for c in range(NCORES):
        wrap = src_rel[c].reshape(M // 16, 16).T.astype(np.int16)
        idx_wrapped[c] = np.tile(wrap, (8, 1))
    dst_f32 = np.empty((NCORES, P, M // P), np.float32)
    dst_bf16 = np.empty((NCORES, P, M // P), np.float32)  # cast later
    for c in range(NCORES):
        dst_f32[c] = dst_loc[c].reshape(M // P, P).T.astype(np.float32)

    # gather pieces: group consecutive (b,w) segments, sum <= GATHER_SLOTS,
    # never splitting a segment; pieces never cross block boundaries.
    pieces = []  # (block, slot0, nslots)
    for b in range(NBLK):
        cur0 = int(seg_off[b, 0])
        cur = 0
        for w in range(W):
            l = int(Lseg[b, w])
            if cur + l > GATHER_SLOTS and cur > 0:
                pieces.append((b, cur0, cur))
                cur0 += cur
                cur = 0
            cur += l
        if cur > 0:
            pieces.append((b, cur0, cur))

    # segments in stream order with chunk counts
    segments = []  # (block, window, slot0, nchunks)
    for b in range(NBLK):
        for w in range(W):
            if Lseg[b, w] > 0:
                segments.append((b, w, int(seg_off[b, w]), int(Lseg[b, w]) // P))

    # bucket column ranges (uniform across cores)
    bucket_ranges = []  # (col0, col1, b); padded rows beyond off[NB] fold
    for b in range(NB):
        if S[b] > 0:
            bucket_ranges.append((int(off[b]), int(off[b + 1]), b))
    # extend last range to R (pad cols; weights of last bucket apply to
    # zero columns -> output stays zero via mask)
    if bucket_ranges:
        c0, c1, b = bucket_ranges[-1]
        bucket_ranges[-1] = (c0, R, b)

    plan = Plan()
    plan.N, plan.E, plan.R, plan.W, plan.M = N, E, R, W, M
    plan.BLK, plan.NBLK = BLK, NBLK
    plan.S, plan.off = S, off
    plan.pieces = pieces
    plan.segments = segments
    plan.bucket_ranges = bucket_ranges
    plan.rows_old = rows_old
    plan.new_global = new_global
    plan.idx_wrapped = idx_wrapped
    plan.dst_f32 = dst_f32
    plan.core_of = core_of
    plan.local = local
    return plan


def _ceil_arr(a, b):
    return (a + b - 1) // b


def _pad2(a, r, c):
    out = np.zeros((r, c), np.float32)
    out[: a.shape[0], : a.shape[1]] = a
    return out


# ---------------------------------------------------------------------------
# Device program
# ---------------------------------------------------------------------------

def _chunks(d):
    """Split feature dim d into partition chunks of <=128."""
    out = []
    s = 0
    while s < d:
        c = min(P, d - s)
        out.append((s, c))
        s += c
    return out


def _col_pieces(c0, c1, maxw=512):
    out = []
    s = c0
    while s < c1:
        e = min(s + maxw, c1)
        out.append((s, e))
        s = e
    return out


def _build(plan):
    STOP = int(os.environ.get("STOP_AFTER", "9"))
    R, W, M = plan.R, plan.W, plan.M
    BLK, NBLK = plan.BLK, plan.NBLK

    nc = bacc.Bacc("TRN2", target_bir_lowering=False, debug=False,
                   num_devices=NCORES)

    # ---- inputs ----
    def din(name, shape, dt):
        return nc.dram_tensor(name, shape, dt, kind="ExternalInput")

    xaug_t = din("xaug", [NCORES * R, 64], F32)        # conv1 gather table
    idx_t = din("idx", [P, M // 16], I16)
    dstf_t = din("dstf", [P, M // P], F32)
    dstb_t = din("dstb", [P, M // P], BF16)
    iota_f = din("iotaf", [P, P], F32)
    iota_b = din("iotab", [P, P], BF16)
    xT_t = din("xT", [4, R], F32)                       # x rows + mask row
    ones_t = din("ones", [8, R], F32)                   # row0 = mask

    w1l_t = din("w1l", [NB, 4, P], F32)
    w1r_t = din("w1r", [NB, 4, P], F32)                 # row3 = bl1
    fc1w_t = din("fc1w", [P, 192], F32)
    b1row_t = din("b1row", [8, 192], F32)               # row0=fc1b, [164]=1
    w2l_t = din("w2l", [NB, 192, 288], F32)
    w2r_t = din("w2r", [NB, 192, 288], F32)             # row164 = bl2
    fc2w_t = din("fc2w", [288, 384], F32)
    b2row_t = din("b2row", [8, 384], F32)               # row0=fc2b, [360]=1
    w3l_t = din("w3l", [NB, 384, 288], BF16)
    w3r_t = din("w3r", [NB, 384, 288], F32)             # row360 = bl3
    fc3w_t = din("fc3w", [288, 192], F32)
    b3row_t = din("b3row", [8, 192], F32)
    l1w_t = din("l1w", [192, 128], F32)
    bl1row_t = din("bl1row", [8, 128], F32)
    l2w_t = din("l2w", [128, 64], F32)
    bl2row_t = din("bl2row", [8, 64], F32)
    ow_t = din("ow", [64, 8], F32)
    borow_t = din("borow", [8, 8], F32)

    outT_t = nc.dram_tensor("outT", [8, R], F32, kind="ExternalOutput")

    # ---- internal DRAM ----
    def dint(name, shape, dt, shared=False):
        return nc.dram_tensor(name, shape, dt, kind="Internal",
                              addr_space="Shared" if shared else "Local")

    c1T_d = dint("c1T", [P, R], F32)
    fc1T_d = [dint("fc1T0", [P, R], F32), dint("fc1T1", [64, R], F32)]
    ag1_in = dint("ag1in", [R, 192], F32)
    table2 = dint("table2", [NCORES * R, 192], F32, shared=True)
    c2T_d = [dint("c2T0", [P, R], F32), dint("c2T1", [P, R], F32),
             dint("c2T2", [32, R], F32)]
    fc2T_d = [dint("fc2T0", [P, R], F32), dint("fc2T1", [P, R], F32),
              dint("fc2T2", [P, R], F32)]
    ag2_in = dint("ag2in", [R, 384], BF16)
    table3 = dint("table3", [NCORES * R, 384], BF16, shared=True)
    c3T_d = [dint("c3T0", [P, R], F32), dint("c3T1", [P, R], F32),
             dint("c3T2", [32, R], F32)]

    ACT = mybir.ActivationFunctionType
    AOP = mybir.AluOpType

    class _StopBuild(Exception):
        pass

    import contextlib
    with tile.TileContext(nc) as tc:
        nc.gpsimd.load_library(mlp_lib)
        with contextlib.suppress(_StopBuild), \
             tc.tile_pool(name="persist", bufs=1) as pp:
            # persistent small tensors
            iotaf = pp.tile([P, P], F32, tag="iotaf")
            nc.sync.dma_start(iotaf[:], iota_f[:, :])
            iotab = pp.tile([P, P], BF16, tag="iotab")
            nc.sync.dma_start(iotab[:], iota_b[:, :])
            dstf = pp.tile([P, M // P], F32, tag="dstf")
            nc.sync.dma_start(dstf[:], dstf_t[:, :])
            dstb = pp.tile([P, M // P], BF16, tag="dstb")
            nc.sync.dma_start(dstb[:], dstb_t[:, :])

            # ============== generic aggregate helper ==============
            def aggregate(table_dram, elem, dt, iota_tile, dst_tile,
                          hT_tiles, hT_chunks, pool, psum_pool):
                for ht, (cs, cw) in zip(hT_tiles, hT_chunks):
                    nc.vector.memset(ht[:], 0.0)
                for (b, s0, ns) in plan.pieces:
                    g = pool.tile([P, (ns // P) * elem], dt, tag="gdst")
                    g3 = g[:].rearrange("p (c e) -> p c e", e=elem)
                    idx_s = pool.tile([P, ns // 16], I16, tag="gidx")
                    nc.sync.dma_start(idx_s[:],
                                      idx_t[:, s0 // 16:(s0 + ns) // 16])
                    nc.gpsimd.dma_gather(
                        g3, table_dram[b * BLK:(b + 1) * BLK, :],
                        idx_s[:], ns, ns, elem, single_packet=False)
                    for (sb, sw, ss0, nch) in plan.segments:
                        if sb != b or ss0 < s0 or ss0 >= s0 + ns:
                            continue
                        psums = []
                        for (cs, cw) in hT_chunks:
                            ps = psum_pool.tile([cw, P], F32, space="PSUM",
                                                tag=f"agg{cs}")
                            psums.append(ps)
                        for j in range(nch):
                            slot = ss0 + j * P
                            col = (slot - s0) // P
                            oh = pool.tile([P, P], dt, tag="oh")
                            nc.vector.tensor_tensor(
                                out=oh[:],
                                in0=dst_tile[:, slot // P:slot // P + 1]
                                .to_broadcast([P, P]),
                                in1=iota_tile[:],
                                op=AOP.is_equal)
                            for k, (cs, cw) in enumerate(hT_chunks):
                                nc.tensor.matmul(
                                    psums[k][:],
                                    lhsT=g3[:, col, cs:cs + cw],
                                    rhs=oh[:],
                                    start=(j == 0), stop=(j == nch - 1))
                        for k, (cs, cw) in enumerate(hT_chunks):
                            dstap = hT_tiles[k][:cw, sw * P:(sw + 1) * P]
                            nc.vector.tensor_tensor(
                                out=dstap, in0=dstap, in1=psums[k][:],
                                op=AOP.add)

            if STOP < 1:
                raise _StopBuild()
            # ================= conv1 =================
            with tc.tile_pool(name="c1h", bufs=1) as hp, \
                 tc.tile_pool(name="c1", bufs=2) as pool:
                h1T = hp.tile([8, R], F32, tag="h1T")
                with tc.tile_pool(name="c1aps", bufs=2, space="PSUM") as psp:
                    aggregate(xaug_t, 64, F32, iotaf, dstf,
                              [h1T], [(0, 8)], pool, psp)
                with tc.tile_pool(name="c1xps", bufs=2, space="PSUM") as psp:
                    for (rc0, rc1, bkt) in plan.bucket_ranges:
                        wl = pool.tile([4, P], F32, tag="w1l")
                        nc.sync.dma_start(wl[:], w1l_t[bkt, :, :])
                        wr = pool.tile([4, P], F32, tag="w1r")
                        nc.sync.dma_start(wr[:], w1r_t[bkt, :, :])
                        for (c0, c1) in _col_pieces(rc0, rc1):
                            cw = c1 - c0
                            xTs = pool.tile([4, 512], F32, tag="xTs")
                            nc.sync.dma_start(xTs[:, :cw], xT_t[0:4, c0:c1])
                            ps = psp.tile([P, 512], F32, space="PSUM",
                                          tag="c1ps")
                            nc.tensor.matmul(ps[:, :cw], lhsT=wl[:],
                                             rhs=h1T[0:4, c0:c1],
                                             start=True, stop=False)
                            nc.tensor.matmul(ps[:, :cw], lhsT=wr[:],
                                             rhs=xTs[0:4, :cw],
                                             start=False, stop=True)
                            ot = pool.tile([P, 512], F32, tag="c1o")
                            nc.scalar.activation(ot[:, :cw], ps[:, :cw],
                                                 ACT.Relu)
                            nc.sync.dma_start(c1T_d[:, c0:c1], ot[:, :cw])

            if STOP < 2:
                raise _StopBuild()
            # ================= fc1 (dual) =================
            with tc.tile_pool(name="f1", bufs=2) as pool, \
                 tc.tile_pool(name="f1ps", bufs=2, space="PSUM") as psp:
                fc1w = pool.tile([P, 192], F32, tag="fc1w")
                nc.sync.dma_start(fc1w[:], fc1w_t[:, :])
                b1row = pool.tile([8, 192], F32, tag="b1row")
                nc.sync.dma_start(b1row[:], b1row_t[:, :])
                for (c0, c1) in _col_pieces(0, R):
                    cw = c1 - c0
                    c1in = pool.tile([P, 512], F32, tag="f1i")
                    nc.sync.dma_start(c1in[:, :cw], c1T_d[:, c0:c1])
                    onesl = pool.tile([8, 512], F32, tag="f1ones")
                    nc.sync.dma_start(onesl[:, :cw], ones_t[:, c0:c1])
                    # (a) transposed: do chunks (128, 64)
                    for ko, (os_, oc) in enumerate([(0, P), (P, 64)]):
                        ps = psp.tile([oc, 512], F32, space="PSUM",
                                      tag=f"f1ps{ko}")
                        nc.tensor.matmul(ps[:, :cw],
                                         lhsT=fc1w[:, os_:os_ + oc],
                                         rhs=c1in[:, :cw],
                                         start=True, stop=False)
                        nc.tensor.matmul(ps[:, :cw],
                                         lhsT=b1row[:, os_:os_ + oc],
                                         rhs=onesl[:, :cw],
                                         start=False, stop=True)
                        ot = pool.tile([oc, 512], F32, tag=f"f1o{ko}")
                        nc.scalar.activation(ot[:, :cw], ps[:, :cw],
                                             ACT.Lrelu, alpha=SLOPE)
                        nc.sync.dma_start(fc1T_d[ko][:oc, c0:c1],
                                          ot[:oc, :cw])
                    # (b) row-major for the gather table
                    for t0 in range(c0, c1, P):
                        j = t0 - c0
                        ps = psp.tile([P, 192], F32, space="PSUM", tag="f1rp")
                        nc.tensor.matmul(ps[:], lhsT=c1in[:, j:j + P],
                                         rhs=fc1w[:], start=True, stop=False)
                        nc.tensor.matmul(ps[:], lhsT=onesl[:, j:j + P],
                                         rhs=b1row[:], start=False, stop=True)
                        rt = pool.tile([P, 192], F32, tag="f1r")
                        nc.scalar.activation(rt[:], ps[:], ACT.Lrelu,
                                             alpha=SLOPE)
                        nc.sync.dma_start(ag1_in[t0:t0 + P, :], rt[:])
                nc.gpsimd.collective_compute(
                    "AllGather", AOP.bypass,
                    replica_groups=[list(range(NCORES))],
                    ins=[ag1_in[:, :]], outs=[table2[:, :]])

            if STOP < 3:
                raise _StopBuild()
            # ================= conv2 =================
            with tc.tile_pool(name="c2h", bufs=1) as hp, \
                 tc.tile_pool(name="c2", bufs=2) as pool:
                h2T = [hp.tile([P, R], F32, tag="h2T0", name="h2T0"),
                       hp.tile([64, R], F32, tag="h2T1", name="h2T1")]
                with tc.tile_pool(name="c2aps", bufs=2, space="PSUM") as psp:
                    aggregate(table2, 192, F32, iotaf, dstf,
                              h2T, [(0, P), (P, 64)], pool, psp)
                in_c = [(0, P), (P, 64)]
                do_chunks = [(0, P), (P, P), (256, 32)]
                with tc.tile_pool(name="c2xps", bufs=2, space="PSUM") as psp:
                    for (rc0, rc1, bkt) in plan.bucket_ranges:
                        wts = {}
                        for ki, (ds, dc) in enumerate(in_c):
                            for ko, (os_, oc) in enumerate(do_chunks):
                                wl = pool.tile([dc, oc], F32,
                                               tag=f"w2l{ki}_{ko}")
                                nc.sync.dma_start(
                                    wl[:],
                                    w2l_t[bkt, ds:ds + dc, os_:os_ + oc])
                                wr = pool.tile([dc, oc], F32,
                                               tag=f"w2r{ki}_{ko}")
                                nc.sync.dma_start(
                                    wr[:],
                                    w2r_t[bkt, ds:ds + dc, os_:os_ + oc])
                                wts[(ki, ko)] = (wl, wr)
                        for (c0, c1) in _col_pieces(rc0, rc1):
                            cw = c1 - c0
                            xts = []
                            for ki, (ds, dc) in enumerate(in_c):
                                t = pool.tile([dc, 512], F32, tag=f"x2l{ki}")
                                nc.sync.dma_start(t[:, :cw],
                                                  fc1T_d[ki][:dc, c0:c1])
                                xts.append(t)
                            for ko, (os_, oc) in enumerate(do_chunks):
                                ps = psp.tile([oc, 512], F32, space="PSUM",
                                              tag=f"c2ps{ko}")
                                for ki, (ds, dc) in enumerate(in_c):
                                    wl, wr = wts[(ki, ko)]
                                    nc.tensor.matmul(
                                        ps[:, :cw], lhsT=wl[:],
                                        rhs=h2T[ki][:dc, c0:c1],
                                        start=(ki == 0), stop=False)
                                    nc.tensor.matmul(
                                        ps[:, :cw], lhsT=wr[:],
                                        rhs=xts[ki][:dc, :cw],
                                        start=False,
                                        stop=(ki == len(in_c) - 1))
                                ot = pool.tile([oc, 512], F32, tag=f"c2o{ko}")
                                nc.scalar.activation(ot[:, :cw], ps[:, :cw],
                                                     ACT.Relu)
                                nc.sync.dma_start(c2T_d[ko][:oc, c0:c1],
                                                  ot[:oc, :cw])

            if STOP < 4:
                raise _StopBuild()
            # ================= fc2 (dual) =================
            with tc.tile_pool(name="f2", bufs=2) as pool, \
                 tc.tile_pool(name="f2ps", bufs=2, space="PSUM") as psp:
                in_chunks = [(0, P), (P, P), (256, 32)]
                do_chunks = [(0, P), (P, P), (256, P)]
                fw = {}
                for ki, (ds, dc) in enumerate(in_chunks):
                    for ko, (os_, oc) in enumerate(do_chunks):
                        t = pool.tile([dc, oc], F32, tag=f"fc2w{ki}_{ko}")
                        nc.sync.dma_start(t[:],
                                          fc2w_t[ds:ds + dc, os_:os_ + oc])
                        fw[(ki, ko)] = t
                fwr = []
                for ki, (ds, dc) in enumerate(in_chunks):
                    t = pool.tile([dc, 384], F32, tag=f"fc2wr{ki}")
                    nc.sync.dma_start(t[:], fc2w_t[ds:ds + dc, :])
                    fwr.append(t)
                b2row = pool.tile([8, 384], F32, tag="b2row")
                nc.sync.dma_start(b2row[:], b2row_t[:, :])
                for (c0, c1) in _col_pieces(0, R):
                    cw = c1 - c0
                    onesl = pool.tile([8, 512], F32, tag="f2ones")
                    nc.sync.dma_start(onesl[:, :cw], ones_t[:, c0:c1])
                    ins = []
                    for ki, (ds, dc) in enumerate(in_chunks):
                        t = pool.tile([dc, 512], F32, tag=f"f2i{ki}")
                        nc.sync.dma_start(t[:, :cw], c2T_d[ki][:dc, c0:c1])
                        ins.append(t)
                    # (a) transposed
                    for ko, (os_, oc) in enumerate(do_chunks):
                        ps = psp.tile([oc, 512], F32, space="PSUM",
                                      tag=f"f2ps{ko}")
                        for ki, (ds, dc) in enumerate(in_chunks):
                            nc.tensor.matmul(ps[:, :cw], lhsT=fw[(ki, ko)][:],
                                             rhs=ins[ki][:dc, :cw],
                                             start=(ki == 0), stop=False)
                        nc.tensor.matmul(ps[:, :cw],
                                         lhsT=b2row[:, os_:os_ + oc],
                                         rhs=onesl[:, :cw],
                                         start=False, stop=True)
                        ot = pool.tile([oc, 512], F32, tag=f"f2o{ko}")
                        nc.scalar.activation(ot[:, :cw], ps[:, :cw],
                                             ACT.Lrelu, alpha=SLOPE)
                        nc.sync.dma_start(fc2T_d[ko][:oc, c0:c1],
                                          ot[:oc, :cw])
                    # (b) row-major bf16 table
                    for t0 in range(c0, c1, P):
                        j = t0 - c0
                        ps = psp.tile([P, 384], F32, space="PSUM", tag="f2rp")
                        for ki, (ds, dc) in enumerate(in_chunks):
                            nc.tensor.matmul(
                                ps[:], lhsT=ins[ki][:dc, j:j + P],
                                rhs=fwr[ki][:],
                                start=(ki == 0), stop=False)
                        nc.tensor.matmul(ps[:], lhsT=onesl[:, j:j + P],
                                         rhs=b2row[:], start=False, stop=True)
                        rt = pool.tile([P, 384], BF16, tag="f2r")
                        nc.scalar.activation(rt[:], ps[:], ACT.Lrelu,
                                             alpha=SLOPE)
                        nc.sync.dma_start(ag2_in[t0:t0 + P, :], rt[:])
                nc.gpsimd.collective_compute(
                    "AllGather", AOP.bypass,
                    replica_groups=[list(range(NCORES))],
                    ins=[ag2_in[:, :]], outs=[table3[:, :]])

            if STOP < 5:
                raise _StopBuild()
            # ================= conv3 =================
            with tc.tile_pool(name="c3h", bufs=1) as hp, \
                 tc.tile_pool(name="c3", bufs=2) as pool:
                h3T = [hp.tile([P, R], BF16, tag="h3T0", name="h3T0"),
                       hp.tile([P, R], BF16, tag="h3T1", name="h3T1"),
                       hp.tile([P, R], BF16, tag="h3T2", name="h3T2")]
                with tc.tile_pool(name="c3aps", bufs=2, space="PSUM") as psp:
                    aggregate(table3, 384, BF16, iotab, dstb,
                              h3T, [(0, P), (P, P), (256, P)], pool, psp)
                in_c = [(0, P), (P, P), (256, P)]
                do_chunks = [(0, P), (P, P), (256, 32)]
                with tc.tile_pool(name="c3xps", bufs=2, space="PSUM") as psp:
                    for (rc0, rc1, bkt) in plan.bucket_ranges:
                        wts = {}
                        for ki, (ds, dc) in enumerate(in_c):
                            for ko, (os_, oc) in enumerate(do_chunks):
                                wl = pool.tile([dc, oc], BF16,
                                               tag=f"w3l{ki}_{ko}")
                                nc.sync.dma_start(
                                    wl[:],
                                    w3l_t[bkt, ds:ds + dc, os_:os_ + oc])
                                wr = pool.tile([dc, oc], F32,
                                               tag=f"w3r{ki}_{ko}")
                                nc.sync.dma_start(
                                    wr[:],
                                    w3r_t[bkt, ds:ds + dc, os_:os_ + oc])
                                wts[(ki, ko)] = (wl, wr)
                        for (c0, c1) in _col_pieces(rc0, rc1):
                            cw = c1 - c0
                            xts = []
                            for ki, (ds, dc) in enumerate(in_c):
                                t = pool.tile([dc, 512], F32, tag=f"x3l{ki}")
                                nc.sync.dma_start(t[:, :cw],
                                                  fc2T_d[ki][:dc, c0:c1])
                                xts.append(t)
                            for ko, (os_, oc) in enumerate(do_chunks):
                                ps = psp.tile([oc, 512], F32, space="PSUM",
                                              tag=f"c3ps{ko}")
                                for ki, (ds, dc) in enumerate(in_c):
                                    wl, wr = wts[(ki, ko)]
                                    nc.tensor.matmul(
                                        ps[:, :cw], lhsT=wl[:],
                                        rhs=h3T[ki][:dc, c0:c1],
                                        start=(ki == 0), stop=False)
                                    nc.tensor.matmul(
                                        ps[:, :cw], lhsT=wr[:],
                                        rhs=xts[ki][:dc, :cw],
                                        start=False,
                                        stop=(ki == len(in_c) - 1))
                                ot = pool.tile([oc, 512], F32, tag=f"c3o{ko}")
                                nc.scalar.activation(ot[:, :cw], ps[:, :cw],
                                                     ACT.Relu)
                                nc.sync.dma_start(c3T_d[ko][:oc, c0:c1],
                                                  ot[:oc, :cw])

            if STOP < 6:
                raise _StopBuild()
            # ========== fused tail: fc3 -> lin1 -> lin2 -> out ==========
            with tc.tile_pool(name="tail", bufs=2) as pool, \
                 tc.tile_pool(name="tailps", bufs=1, space="PSUM") as psp:
                in_chunks = [(0, P), (P, P), (256, 32)]
                do3 = [(0, P), (P, 64)]
                fw3 = {}
                for ki, (ds, dc) in enumerate(in_chunks):
                    for ko, (os_, oc) in enumerate(do3):
                        t = pool.tile([dc, oc], F32, tag=f"fc3w{ki}_{ko}",
                                      name=f"fc3w{ki}_{ko}")
                        nc.sync.dma_start(t[:],
                                          fc3w_t[ds:ds + dc, os_:os_ + oc])
                        fw3[(ki, ko)] = t
                b3row = pool.tile([8, 192], F32, tag="b3row")
                nc.sync.dma_start(b3row[:], b3row_t[:, :])
                w1 = {}
                for ki, (ds, dc) in enumerate([(0, P), (P, 64)]):
                    t = pool.tile([dc, P], F32, tag=f"l1w{ki}",
                                  name=f"l1w{ki}")
                    nc.sync.dma_start(t[:], l1w_t[ds:ds + dc, :])
                    w1[ki] = t
                br1 = pool.tile([8, P], F32, tag="bl1row")
                nc.sync.dma_start(br1[:], bl1row_t[:, :])
                wt2 = pool.tile([P, 64], F32, tag="l2w")
                nc.sync.dma_start(wt2[:], l2w_t[:, :])
                br2 = pool.tile([8, 64], F32, tag="bl2row")
                nc.sync.dma_start(br2[:], bl2row_t[:, :])
                wo = pool.tile([64, 8], F32, tag="ow")
                nc.sync.dma_start(wo[:], ow_t[:, :])
                bro = pool.tile([8, 8], F32, tag="borow")
                nc.sync.dma_start(bro[:], borow_t[:, :])
                for (c0, c1) in _col_pieces(0, R):
                    cw = c1 - c0
                    onesl = pool.tile([8, 512], F32, tag="tones")
                    nc.sync.dma_start(onesl[:, :cw], ones_t[:, c0:c1])
                    ins = []
                    for ki, (ds, dc) in enumerate(in_chunks):
                        t = pool.tile([dc, 512], F32, tag=f"f3i{ki}",
                                      name=f"f3i{ki}")
                        nc.sync.dma_start(t[:, :cw], c3T_d[ki][:dc, c0:c1])
                        ins.append(t)
                    # fc3 -> f3o tiles (192 = 128 + 64), Lrelu
                    f3o = []
                    for ko, (os_, oc) in enumerate(do3):
                        ps = psp.tile([oc, 512], F32, space="PSUM",
                                      tag=f"f3ps{ko}")
                        for ki, (ds, dc) in enumerate(in_chunks):
                            nc.tensor.matmul(ps[:, :cw],
                                             lhsT=fw3[(ki, ko)][:],
                                             rhs=ins[ki][:dc, :cw],
                                             start=(ki == 0), stop=False)
                        nc.tensor.matmul(ps[:, :cw],
                                         lhsT=b3row[:, os_:os_ + oc],
                                         rhs=onesl[:, :cw],
                                         start=False, stop=True)
                        ot = pool.tile([oc, 512], F32, tag=f"f3o{ko}",
                                       name=f"f3o{ko}")
                        nc.scalar.activation(ot[:, :cw], ps[:, :cw],
                                             ACT.Lrelu, alpha=SLOPE)
                        f3o.append(ot)
                    # lin1
                    ps1 = psp.tile([P, 512], F32, space="PSUM", tag="l1ps")
                    for ki, (ds, dc) in enumerate([(0, P), (P, 64)]):
                        nc.tensor.matmul(ps1[:, :cw], lhsT=w1[ki][:],
                                         rhs=f3o[ki][:dc, :cw],
                                         start=(ki == 0), stop=False)
                    nc.tensor.matmul(ps1[:, :cw], lhsT=br1[:],
                                     rhs=onesl[:, :cw],
                                     start=False, stop=True)
                    l1o = pool.tile([P, 512], F32, tag="l1o")
                    nc.scalar.activation(l1o[:, :cw], ps1[:, :cw], ACT.Copy)
                    # lin2
                    ps2 = psp.tile([64, 512], F32, space="PSUM", tag="l2ps")
                    nc.tensor.matmul(ps2[:, :cw], lhsT=wt2[:],
                                     rhs=l1o[:, :cw], start=True, stop=False)
                    nc.tensor.matmul(ps2[:, :cw], lhsT=br2[:],
                                     rhs=onesl[:, :cw],
                                     start=False, stop=True)
                    l2o = pool.tile([64, 512], F32, tag="l2o")
                    nc.scalar.activation(l2o[:, :cw], ps2[:, :cw], ACT.Copy)
                    # out + sigmoid
                    ps3 = psp.tile([8, 512], F32, space="PSUM", tag="ops")
                    nc.tensor.matmul(ps3[:, :cw], lhsT=wo[:],
                                     rhs=l2o[:, :cw], start=True, stop=False)
                    nc.tensor.matmul(ps3[:, :cw], lhsT=bro[:],
                                     rhs=onesl[:, :cw],
                                     start=False, stop=True)
                    oo = pool.tile([8, 512], F32, tag="oout")
                    nc.scalar.activation(oo[:, :cw], ps3[:, :cw], ACT.Sigmoid)
                    nc.sync.dma_start(outT_t[:, c0:c1], oo[:, :cw])

    nc.compile()
    return nc


# ---------------------------------------------------------------------------
# kernel entry
# ---------------------------------------------------------------------------

def _pack_inputs(plan, x, Wl1, Wr1, bl1, fc1W, fc1b, Wl2, Wr2, bl2, fc2W,
                 fc2b, Wl3, Wr3, bl3, fc3W, fc3b, lin1W, lin1b, lin2W, lin2b,
                 outW, outb):
    R, M = plan.R, plan.M
    N = plan.N

    # conv1 gather table: [8R, 64] rows = [x0,x1,x2,1, 0...]
    xaug = np.zeros((NCORES * R, 64), np.float32)
    xaug[plan.new_global, :3] = x
    xaug[plan.new_global, 3] = 1.0

    # per-core xT [4, R] (x rows + mask) and ones [8, R] (row0 = mask)
    xT = np.zeros((NCORES, 4, R), np.float32)
    ones = np.zeros((NCORES, 8, R), np.float32)
    xT[plan.core_of, :3, plan.local] = x
    xT[plan.core_of, 3, plan.local] = 1.0
    ones[plan.core_of, 0, plan.local] = 1.0

    iota_f = np.tile(np.arange(P, dtype=np.float32), (P, 1))

    def brow(b, width, mask_col=None):
        out = np.zeros((8, width), np.float32)
        out[0, : len(b)] = b
        if mask_col is not None:
            out[0, mask_col] = 1.0
        return out

    w1l = np.zeros((NB, 4, P), np.float32)
    w1l[:, :3, :] = Wl1
    w1r = np.zeros((NB, 4, P), np.float32)
    w1r[:, :3, :] = Wr1
    w1r[:, 3, :] = bl1

    w2l = np.zeros((NB, 192, 288), np.float32)
    w2l[:, :164, :286] = Wl2
    w2r = np.zeros((NB, 192, 288), np.float32)
    w2r[:, :164, :286] = Wr2
    w2r[:, 164, :286] = bl2

    w3l = np.zeros((NB, 384, 288), np.float32)
    w3l[:, :360, :286] = Wl3
    w3r = np.zeros((NB, 384, 288), np.float32)
    w3r[:, :360, :286] = Wr3
    w3r[:, 360, :286] = bl3

    common = {
        "xaug": xaug,
        "iotaf": iota_f,
        "iotab": iota_f.astype(np.float32),  # cast to bf16 below
        "w1l": w1l, "w1r": w1r,
        "fc1w": _pad2(fc1W, P, 192),
        "b1row": brow(fc1b, 192, mask_col=164),
        "w2l": w2l, "w2r": w2r,
        "fc2w": _pad2(fc2W, 288, 384),
        "b2row": brow(fc2b, 384, mask_col=360),
        "w3l": w3l, "w3r": w3r,
        "fc3w": _pad2(fc3W, 288, 192),
        "b3row": brow(fc3b, 192),
        "l1w": _pad2(lin1W, 192, P),
        "bl1row": brow(lin1b, P),
        "l2w": _pad2(lin2W, P, 64),
        "bl2row": brow(lin2b, 64),
        "ow": _pad2(outW, 64, 8),
        "borow": brow(outb, 8),
    }
    import ml_dtypes
    in_maps = []
    for c in range(NCORES):
        m = dict(common)
        m["iotab"] = iota_f.astype(ml_dtypes.bfloat16)
        m["w3l"] = w3l.astype(ml_dtypes.bfloat16)
        m["idx"] = plan.idx_wrapped[c]
        m["dstf"] = plan.dst_f32[c]
        m["dstb"] = plan.dst_f32[c].astype(ml_dtypes.bfloat16)
        m["xT"] = xT[c]
        m["ones"] = ones[c]
        in_maps.append(m)
    return in_maps


_CACHE = {}


def kernel(**inputs):
    x = np.ascontiguousarray(np.asarray(inputs["x"], dtype=np.float32))
    edge_index = np.asarray(inputs["edge_index"], dtype=np.int64)

    plan = _preprocess(x, edge_index)
    in_maps = _pack_inputs(
        plan, x,
        np.asarray(inputs["Wl1"], np.float32),
        np.asarray(inputs["Wr1"], np.float32),
        np.asarray(inputs["bl1"], np.float32),
        np.asarray(inputs["fc1W"], np.float32),
        np.asarray(inputs["fc1b"], np.float32),
        np.asarray(inputs["Wl2"], np.float32),
        np.asarray(inputs["Wr2"], np.float32),
        np.asarray(inputs["bl2"], np.float32),
        np.asarray(inputs["fc2W"], np.float32),
        np.asarray(inputs["fc2b"], np.float32),
        np.asarray(inputs["Wl3"], np.float32),
        np.asarray(inputs["Wr3"], np.float32),
        np.asarray(inputs["bl3"], np.float32),
        np.asarray(inputs["fc3W"], np.float32),
        np.asarray(inputs["fc3b"], np.float32),
        np.asarray(inputs["lin1W"], np.float32),
        np.asarray(inputs["lin1b"], np.float32),
        np.asarray(inputs["lin2W"], np.float32),
        np.asarray(inputs["lin2b"], np.float32),
        np.asarray(inputs["outW"], np.float32),
        np.asarray(inputs["outb"], np.float32),
    )

    nc = _build(plan)
    res = bass_utils.run_bass_kernel_spmd(
        nc, in_maps, core_ids=list(range(NCORES)))
    kernel._last_results = res

    out = np.empty((plan.N, 6), np.float32)
    for c in range(NCORES):
        oT = np.asarray(res.results[c]["outT"])  # [8, R]
        rows = plan.rows_old[c]
        valid = rows >= 0
        out[rows[valid]] = oT[:6, valid].T
    return out




# revision 10
# speedup vs baseline: 80.3555x; 80.3555x over previous
"""Trainium2 Bass kernel for nn_GCNConvNet (MFConv GNN, N=100k, E=1.6M).

Strategy (8 NeuronCores, SPMD):
  - Nodes renumbered on host: dealt round-robin per degree-bucket so every
    core owns R rows laid out bucket-contiguously (uniform bucket offsets
    across cores -> one shared program). Pad rows stay exactly zero.
  - Activations live row-major bf16 in DRAM. Per-layer tables for the edge
    gather are built with on-device AllGather of each core's rows.
  - Aggregation h = A @ x runs in a For_i hardware loop over 256-row dst
    windows: dma_gather of src rows (bf16) -> one-hot matrices on DVE ->
    TensorE matmuls accumulate window columns in f32 PSUM -> merged into
    SBUF-resident h^T (bf16).
  - Weight matmuls run node-tile-major: psum[128 nodes, feat_out] with
    lhsT = h^T (SBUF) / x^T (via transpose-mode dma_gather of the row-major
    DRAM activations, fetched lazily per 2048-node supertile); per-bucket
    weights; bias applied via a mask-row matmul so pad rows stay zero.
  - The fc3->lin1->lin2->out tail runs in transposed orientation per
    512-column piece, ending in outT [8, R] f32 per core.
  - Plan + program + jitted executable are cached per edge-index hash;
    repeat calls only repack inputs and rerun.
"""

import hashlib
import math
import os
import sys

sys.path.insert(0, "/opt/trn_rl_repo")

import numpy as np
import ml_dtypes

import concourse.bacc as bacc
import concourse.bass as bass
from concourse.bass import ds
import concourse.mybir as mybir
import concourse.tile as tile
from concourse import bass_utils

F32 = mybir.dt.float32
BF16 = mybir.dt.bfloat16
I16 = mybir.dt.int16
ACT = mybir.ActivationFunctionType
AOP = mybir.AluOpType
BF = ml_dtypes.bfloat16

NCORES = 8
P = 128
WIN = 256          # dst rows per aggregation window
MAX_DEG = 10
NB = MAX_DEG + 1
SLOPE = 0.01
GPIECE = 2048      # transpose-gather supertile (nodes)


def _ceil(a, b):
    return (a + b - 1) // b


# ---------------------------------------------------------------------------
# Host-side preprocessing (depends only on edge_index / N)
# ---------------------------------------------------------------------------

class Plan:
    pass


def _preprocess(N, src, dst):
    deg = np.bincount(dst, minlength=N).astype(np.int64)
    bucket = np.minimum(deg, MAX_DEG)

    order = np.argsort(bucket, kind="stable")
    pos = np.empty(N, np.int64)
    pos[order] = np.arange(N)
    core_of = pos % NCORES

    cnt_b = np.bincount(bucket, minlength=NB)
    seg_start = np.zeros(NB + 1, np.int64)
    seg_start[1:] = np.cumsum(cnt_b)
    S = _ceil(_ceil(cnt_b, NCORES), P) * P   # per-(core,bucket), 128-aligned
    off = np.zeros(NB + 1, np.int64)
    off[1:] = np.cumsum(S)
    R = int(math.ceil((off[NB] + 1) / WIN) * WIN)
    assert 2 * R <= 32767, f"block size {2*R} exceeds int16"

    local = off[bucket] + (pos - seg_start[bucket]) // NCORES
    new_global = core_of * R + local

    rows_old = np.full((NCORES, R), -1, np.int64)
    rows_old[core_of, local] = np.arange(N)

    # ---- edge slot streams ----
    BLK = 2 * R
    NBLK = 4
    W = R // WIN                       # windows per core
    ns = new_global[src]
    nd = new_global[dst]
    ecore = nd // R
    eldst = nd % R
    eblk = ns // BLK
    egrel = ns % BLK
    ewin = eldst // WIN

    # uniform L per (block, window) cell across cores:
    key = (ecore * NBLK + eblk) * W + ewin
    cnt = np.bincount(key, minlength=NCORES * NBLK * W).reshape(
        NCORES, NBLK, W)
    L = int(_ceil(max(1, int(cnt.max())), P) * P)
    M = NBLK * W * L

    # slot of edge e (in its core's stream): cell offset + rank within cell
    eorder = np.lexsort((ns, ewin, eblk, ecore))
    k2 = key[eorder]
    E = len(src)
    group_starts = np.flatnonzero(np.r_[True, k2[1:] != k2[:-1]])
    lens = np.diff(np.r_[group_starts, E])
    rank = np.arange(E) - np.repeat(group_starts, lens)
    cell = (eblk[eorder] * W + ewin[eorder])
    slot = cell * L + rank
    assert int(rank.max()) < L

    zrel = int(off[NB])                # relative zero/pad row inside a block
    gidx = np.full((NCORES, M), zrel, np.int16)
    dloc = np.zeros((NCORES, M), np.int16)
    ec = ecore[eorder]
    gidx[ec, slot] = egrel[eorder].astype(np.int16)
    dloc[ec, slot] = (eldst[eorder] % WIN).astype(np.int16)

    # wrapped idx stream [16, M/16]; dst window values [128, M/128] bf16
    gidx_w = np.ascontiguousarray(
        gidx.reshape(NCORES, M // 16, 16).transpose(0, 2, 1))
    dst_w = np.ascontiguousarray(
        dloc.reshape(NCORES, M // P, P).transpose(0, 2, 1)).astype(BF)

    # sequential idx for transpose-gathers [16, R/16]
    seq = np.arange(R, dtype=np.int16).reshape(R // 16, 16).T
    seq = np.ascontiguousarray(seq)

    # bucket of each 128-node tile (bucket sizes are 128-aligned)
    tiles = []
    for t in range(R // P):
        bkt = int(np.searchsorted(off[1:NB + 1], t * P, side="right"))
        tiles.append(min(bkt, NB - 1))

    plan = Plan()
    plan.N, plan.E, plan.R, plan.W, plan.M, plan.L = N, E, R, W, M, L
    plan.BLK, plan.NBLK = BLK, NBLK
    plan.S, plan.off = S, off
    plan.rows_old = rows_old
    plan.core_of, plan.local = core_of, local
    plan.gidx_w, plan.dst_w, plan.seq = gidx_w, dst_w, seq
    plan.tiles = tiles
    return plan


def _pad2(a, r, c):
    out = np.zeros((r, c), np.float32)
    out[: a.shape[0], : a.shape[1]] = a
    return out


def _pad3(a, n, r, c):
    out = np.zeros((n, r, c), np.float32)
    out[:, : a.shape[1], : a.shape[2]] = a
    return out


# ---------------------------------------------------------------------------
# Device program
# ---------------------------------------------------------------------------

def _build(plan):
    R, W, M, L = plan.R, plan.W, plan.M, plan.L
    BLK, NBLK = plan.BLK, plan.NBLK
    LC = L // P                         # gather chunks per cell
    NT = R // P                         # node tiles
    WSH = plan.WSH                      # weight-blob shard elems (bf16)

    nc = bacc.Bacc("TRN2", target_bir_lowering=False, debug=False,
                   num_devices=NCORES)

    def din(name, shape, dt):
        return nc.dram_tensor(name, shape, dt, kind="ExternalInput")

    def dint(name, shape, dt, shared=False):
        return nc.dram_tensor(name, shape, dt, kind="Internal",
                              addr_space="Shared" if shared else "Local")

    x_in = din("x_in", [R, P], BF16)
    gidx_t = din("gidx", [16, M // 16], I16)
    dstv_t = din("dstv", [P, M // P], BF16)
    seq_t = din("seq", [16, R // 16], I16)
    mask_t = din("mask", [1, R], BF16)
    iota_t = din("iota", [P, WIN], BF16)
    wsh_t = din("wsh", [1, WSH], BF16)

    w1l_t = din("w1l", [4, NB * P], BF16)
    w1r_t = din("w1r", [4, NB * P], BF16)
    b1_t = din("b1", [1, NB * P], BF16)
    fc1w_t = din("fc1w", [P, 256], BF16)
    fc1b_t = din("fc1b", [1, 256], BF16)
    b2_t = din("b2", [1, NB * 384], BF16)
    fc2w_t = din("fc2w", [384, 384], BF16)
    fc2b_t = din("fc2b", [1, 384], BF16)
    b3_t = din("b3", [1, NB * 384], BF16)
    fc3w_t = din("fc3w", [384, 256], BF16)
    fc3b_t = din("fc3b", [1, 256], BF16)
    l1w_t = din("l1w", [192, P], BF16)
    l1b_t = din("l1b", [1, P], BF16)
    l2w_t = din("l2w", [P, 64], BF16)
    l2b_t = din("l2b", [1, 64], BF16)
    ow_t = din("ow", [64, 8], BF16)
    ob_t = din("ob", [1, 8], BF16)

    outT_t = nc.dram_tensor("outT", [8, R], F32, kind="ExternalOutput")

    table1 = dint("table1", [NCORES * R, P], BF16, shared=True)
    x_loc = dint("x_loc", [R, P], BF16)
    wsh_loc = dint("wsh_loc", [1, WSH], BF16)
    wblob = dint("wblob", [NCORES, WSH], BF16, shared=True)
    c1_d = dint("c1", [R, P], BF16)
    ag1_d = dint("ag1", [R, 256], BF16)
    table2 = dint("table2", [NCORES * R, 256], BF16, shared=True)
    c2_d = dint("c2", [R, 384], BF16)
    ag2_d = dint("ag2", [R, 384], BF16)
    table3 = dint("table3", [NCORES * R, 384], BF16, shared=True)
    c3_d = dint("c3", [R, 384], BF16)

    groups = [list(range(NCORES))]
    wblob_f = wblob[:, :].rearrange("a b -> (a b)")
    STOP = int(os.environ.get("STOP_AFTER", "99"))

    class _StopBuild(Exception):
        pass

    import contextlib
    with tile.TileContext(nc) as tc:
        with contextlib.suppress(_StopBuild), \
             tc.tile_pool(name="persist", bufs=1) as pp:
            seq = pp.tile([P, R // 16], I16, tag="seq")
            for k in range(8):
                nc.sync.dma_start(seq[16 * k:16 * (k + 1), :], seq_t[:, :])
            mask = pp.tile([1, R], BF16, tag="mask")
            nc.sync.dma_start(mask[:], mask_t[:, :])
            iota = pp.tile([P, WIN], BF16, tag="iota")
            nc.sync.dma_start(iota[:], iota_t[:, :])

            nc.sync.dma_start(x_loc[:, :], x_in[:, :])
            nc.sync.dma_start(wsh_loc[:, :], wsh_t[:, :])
            nc.gpsimd.collective_compute(
                "AllGather", AOP.bypass, replica_groups=groups,
                ins=[x_loc[:, :]], outs=[table1[:, :]])
            nc.gpsimd.collective_compute(
                "AllGather", AOP.bypass, replica_groups=groups,
                ins=[wsh_loc[:, :]], outs=[wblob[:, :]])
            if STOP < 2:
                raise _StopBuild()

            # ---- helpers ----
            def aggregate(table, elem, hT, pool, psp):
                """h^T (SBUF bf16 tiles, [128, R] each) += table[src] rows.
                For_i over W windows; static python over 4 src blocks."""
                nchunk = len(hT)
                for ht in hT:
                    nc.vector.memset(ht[:], 0.0)
                gidx = pool.tile([P, M // 16], I16, tag="gidx")
                for k in range(8):
                    nc.sync.dma_start(gidx[16 * k:16 * (k + 1), :],
                                      gidx_t[:, :])
                dstv = pool.tile([P, M // P], BF16, tag="dstv")
                nc.sync.dma_start(dstv[:], dstv_t[:, :])
                g_tiles = [pool.tile([P, LC * elem], BF16, tag=f"g{b}",
                                     name=f"g{b}")
                           for b in range(NBLK)]
                with tc.For_i(0, W, 1) as w:
                    g3s = []
                    for b in range(NBLK):
                        g3 = g_tiles[b][:].rearrange(
                            "p (c e) -> p c e", e=elem)
                        col16 = b * W * (L // 16)
                        nc.gpsimd.dma_gather(
                            g3, table[b * BLK:(b + 1) * BLK, :],
                            gidx[:, ds(col16 + w * (L // 16), L // 16)],
                            L, L, elem, single_packet=False)
                        g3s.append(g3)
                    pss = [psp.tile([P, WIN], F32, space="PSUM",
                                    tag=f"ps{k}", name=f"ps{k}")
                           for k in range(nchunk)]
                    for b in range(NBLK):
                        for j in range(LC):
                            oh = pool.tile([P, WIN], BF16, tag=f"oh{b}")
                            nc.vector.tensor_tensor(
                                out=oh[:],
                                in0=dstv[:, ds(b * W * LC + w * LC + j, 1)]
                                .to_broadcast([P, WIN]),
                                in1=iota[:], op=AOP.is_equal)
                            for k in range(nchunk):
                                cw = min(P, elem - P * k)
                                nc.tensor.matmul(
                                    pss[k][:cw, :],
                                    lhsT=g3s[b][:, j, P * k:P * k + cw],
                                    rhs=oh[:],
                                    start=(b == 0 and j == 0),
                                    stop=(b == NBLK - 1 and j == LC - 1))
                    for k in range(nchunk):
                        cw = min(P, elem - P * k)
                        dap = hT[k][:cw, ds(w * WIN, WIN)]
                        nc.vector.tensor_tensor(
                            out=dap, in0=dap, in1=pss[k][:cw, :],
                            op=AOP.add)

            def gtr_piece(src_d, elem, pool, tag, s0, n):
                """transpose-gather rows [s0, s0+n) -> [128, elem/128, n]."""
                nch = elem // P
                t = pool.tile([P, nch * n], BF16, tag=tag)
                t3 = t[:].rearrange("p (c n) -> p c n", n=n)
                nc.gpsimd.dma_gather(
                    t3, src_d[:, :], seq[:, s0 // 16:(s0 + n) // 16],
                    n, n, elem, transpose=True, single_packet=False)
                return t3

            def supertiles():
                for s0 in range(0, R, GPIECE):
                    yield s0, min(GPIECE, R - s0)

            if STOP < 3:
                raise _StopBuild()
            # ================= conv1 =================
            with tc.tile_pool(name="c1h", bufs=1) as hp, \
                 tc.tile_pool(name="c1", bufs=2) as pool, \
                 tc.tile_pool(name="c1ps", bufs=2, space="PSUM") as psp:
                h1T = [hp.tile([P, R], BF16, tag="h1T", name="h1T")]
                if STOP >= 4:
                    aggregate(table1, P, h1T, pool, psp)
                else:
                    nc.vector.memset(h1T[0][:], 0.0)
                if STOP < 5:
                    raise _StopBuild()
                w1l = pool.tile([4, NB * P], BF16, tag="w1l")
                nc.sync.dma_start(w1l[:], w1l_t[:, :])
                w1r = pool.tile([4, NB * P], BF16, tag="w1r")
                nc.sync.dma_start(w1r[:], w1r_t[:, :])
                b1 = pool.tile([1, NB * P], BF16, tag="b1")
                nc.sync.dma_start(b1[:], b1_t[:, :])
                for s0, n in supertiles():
                    xT = gtr_piece(x_in, P, pool, "x1T", s0, n)
                    for t in range(s0 // P, (s0 + n) // P):
                        j0 = t * P - s0
                        ps = psp.tile([P, P], F32, space="PSUM", tag="c1ps")
                        bkt = plan.tiles[t]
                        nc.tensor.matmul(
                            ps[:], lhsT=h1T[0][0:4, t * P:(t + 1) * P],
                            rhs=w1l[:, bkt * P:(bkt + 1) * P],
                            start=True, stop=False)
                        nc.tensor.matmul(
                            ps[:], lhsT=xT[0:4, 0, j0:j0 + P],
                            rhs=w1r[:, bkt * P:(bkt + 1) * P],
                            start=False, stop=False)
                        nc.tensor.matmul(
                            ps[:], lhsT=mask[0:1, t * P:(t + 1) * P],
                            rhs=b1[0:1, bkt * P:(bkt + 1) * P],
                            start=False, stop=True)
                        ot = pool.tile([P, P], BF16, tag="c1o")
                        nc.scalar.activation(ot[:], ps[:], ACT.Relu)
                        nc.sync.dma_start(c1_d[t * P:(t + 1) * P, :], ot[:])

            if STOP < 6:
                raise _StopBuild()
            # ================= fc1 =================
            with tc.tile_pool(name="f1", bufs=2) as pool, \
                 tc.tile_pool(name="f1ps", bufs=2, space="PSUM") as psp:
                fw = pool.tile([P, 256], BF16, tag="fc1w")
                nc.sync.dma_start(fw[:], fc1w_t[:, :])
                fb = pool.tile([1, 256], BF16, tag="fc1b")
                nc.sync.dma_start(fb[:], fc1b_t[:, :])
                for s0, n in supertiles():
                    cT = gtr_piece(c1_d, P, pool, "c1T", s0, n)
                    for t in range(s0 // P, (s0 + n) // P):
                        j0 = t * P - s0
                        ps = psp.tile([P, 256], F32, space="PSUM", tag="f1ps")
                        nc.tensor.matmul(ps[:], lhsT=cT[:, 0, j0:j0 + P],
                                         rhs=fw[:], start=True, stop=False)
                        nc.tensor.matmul(ps[:],
                                         lhsT=mask[0:1, t * P:(t + 1) * P],
                                         rhs=fb[:], start=False, stop=True)
                        ot = pool.tile([P, 256], BF16, tag="f1o")
                        nc.scalar.activation(ot[:], ps[:], ACT.Lrelu,
                                             alpha=SLOPE)
                        nc.sync.dma_start(ag1_d[t * P:(t + 1) * P, :], ot[:])
                nc.gpsimd.collective_compute(
                    "AllGather", AOP.bypass, replica_groups=groups,
                    ins=[ag1_d[:, :]], outs=[table2[:, :]])

            if STOP < 7:
                raise _StopBuild()
            # ================= conv2 =================
            with tc.tile_pool(name="c2h", bufs=1) as hp, \
                 tc.tile_pool(name="c2", bufs=2) as pool, \
                 tc.tile_pool(name="c2ps", bufs=2, space="PSUM") as psp:
                h2T = [hp.tile([P, R], BF16, tag="h2T0", name="h2T0"),
                       hp.tile([P, R], BF16, tag="h2T1", name="h2T1")]
                aggregate(table2, 256, h2T, pool, psp)
                wt = {}
                o = 0
                for bkt in range(NB):
                    for side in range(2):
                        for ki, kk in enumerate((P, 64)):
                            t_ = pool.tile([kk, 384], BF16,
                                           tag=f"w2_{bkt}_{side}_{ki}")
                            nc.sync.dma_start(
                                t_[:],
                                wblob_f[o:o + kk * 384].rearrange(
                                    "(a b) -> a b", b=384))
                            wt[(bkt, side, ki)] = t_
                            o += kk * 384
                b2 = pool.tile([1, NB * 384], BF16, tag="b2")
                nc.sync.dma_start(b2[:], b2_t[:, :])
                for s0, n in supertiles():
                    xT = gtr_piece(ag1_d, 256, pool, "x2T", s0, n)
                    for t in range(s0 // P, (s0 + n) // P):
                        j0 = t * P - s0
                        ps = psp.tile([P, 384], F32, space="PSUM", tag="c2ps")
                        bkt = plan.tiles[t]
                        for ki, kk in enumerate((P, 64)):
                            nc.tensor.matmul(
                                ps[:],
                                lhsT=h2T[ki][:kk, t * P:(t + 1) * P],
                                rhs=wt[(bkt, 0, ki)][:],
                                start=(ki == 0), stop=False)
                            nc.tensor.matmul(
                                ps[:], lhsT=xT[:kk, ki, j0:j0 + P],
                                rhs=wt[(bkt, 1, ki)][:],
                                start=False, stop=False)
                        nc.tensor.matmul(
                            ps[:], lhsT=mask[0:1, t * P:(t + 1) * P],
                            rhs=b2[0:1, bkt * 384:(bkt + 1) * 384],
                            start=False, stop=True)
                        ot = pool.tile([P, 384], BF16, tag="c2o")
                        nc.scalar.activation(ot[:], ps[:], ACT.Relu)
                        nc.sync.dma_start(c2_d[t * P:(t + 1) * P, :], ot[:])

            if STOP < 8:
                raise _StopBuild()
            # ================= fc2 =================
            with tc.tile_pool(name="f2", bufs=2) as pool, \
                 tc.tile_pool(name="f2ps", bufs=2, space="PSUM") as psp:
                fws = []
                for ki, (p0, kk) in enumerate(((0, P), (P, P), (2 * P, 32))):
                    t_ = pool.tile([kk, 384], BF16, tag=f"fc2w{ki}")
                    nc.sync.dma_start(t_[:], fc2w_t[p0:p0 + kk, :])
                    fws.append(t_)
                fb = pool.tile([1, 384], BF16, tag="fc2b")
                nc.sync.dma_start(fb[:], fc2b_t[:, :])
                for s0, n in supertiles():
                    cT = gtr_piece(c2_d, 384, pool, "c2T", s0, n)
                    for t in range(s0 // P, (s0 + n) // P):
                        j0 = t * P - s0
                        ps = psp.tile([P, 384], F32, space="PSUM", tag="f2ps")
                        for ki, (c, kk) in enumerate(((0, P), (1, P),
                                                      (2, 32))):
                            nc.tensor.matmul(
                                ps[:], lhsT=cT[:kk, c, j0:j0 + P],
                                rhs=fws[ki][:], start=(ki == 0), stop=False)
                        nc.tensor.matmul(ps[:],
                                         lhsT=mask[0:1, t * P:(t + 1) * P],
                                         rhs=fb[:], start=False, stop=True)
                        ot = pool.tile([P, 384], BF16, tag="f2o")
                        nc.scalar.activation(ot[:], ps[:], ACT.Lrelu,
                                             alpha=SLOPE)
                        nc.sync.dma_start(ag2_d[t * P:(t + 1) * P, :], ot[:])
                nc.gpsimd.collective_compute(
                    "AllGather", AOP.bypass, replica_groups=groups,
                    ins=[ag2_d[:, :]], outs=[table3[:, :]])

            if STOP < 9:
                raise _StopBuild()
            # ================= conv3 =================
            with tc.tile_pool(name="c3h", bufs=1) as hp, \
                 tc.tile_pool(name="c3", bufs=2) as pool, \
                 tc.tile_pool(name="c3ps", bufs=2, space="PSUM") as psp:
                h3T = [hp.tile([P, R], BF16, tag=f"h3T{k}", name=f"h3T{k}")
       for k in range(3)]
                aggregate(table3, 384, h3T, pool, psp)
                wt = {}
                o = plan.W3OFF
                for bkt in range(NB):
                    for side in range(2):
                        for ki in range(3):
                            t_ = pool.tile([P, 384], BF16,
                                           tag=f"w3_{bkt}_{side}_{ki}")
                            nc.sync.dma_start(
                                t_[:],
                                wblob_f[o:o + P * 384].rearrange(
                                    "(a b) -> a b", b=384))
                            wt[(bkt, side, ki)] = t_
                            o += P * 384
                b3 = pool.tile([1, NB * 384], BF16, tag="b3")
                nc.sync.dma_start(b3[:], b3_t[:, :])
                for s0, n in supertiles():
                    xT = gtr_piece(ag2_d, 384, pool, "x3T", s0, n)
                    for t in range(s0 // P, (s0 + n) // P):
                        j0 = t * P - s0
                        ps = psp.tile([P, 384], F32, space="PSUM", tag="c3ps")
                        bkt = plan.tiles[t]
                        for ki in range(3):
                            nc.tensor.matmul(
                                ps[:],
                                lhsT=h3T[ki][:, t * P:(t + 1) * P],
                                rhs=wt[(bkt, 0, ki)][:],
                                start=(ki == 0), stop=False)
                            nc.tensor.matmul(
                                ps[:], lhsT=xT[:, ki, j0:j0 + P],
                                rhs=wt[(bkt, 1, ki)][:],
                                start=False, stop=False)
                        nc.tensor.matmul(
                            ps[:], lhsT=mask[0:1, t * P:(t + 1) * P],
                            rhs=b3[0:1, bkt * 384:(bkt + 1) * 384],
                            start=False, stop=True)
                        ot = pool.tile([P, 384], BF16, tag="c3o")
                        nc.scalar.activation(ot[:], ps[:], ACT.Relu)
                        nc.sync.dma_start(c3_d[t * P:(t + 1) * P, :], ot[:])

            if STOP < 10:
                raise _StopBuild()
            # ====== tail: fc3 -> lin1 -> lin2 -> out (transposed) ======
            with tc.tile_pool(name="tl", bufs=2) as pool, \
                 tc.tile_pool(name="tlps", bufs=1, space="PSUM") as psp:
                f3w = []
                for ki, (p0, kk) in enumerate(((0, P), (P, P), (2 * P, 32))):
                    t_ = pool.tile([kk, 256], BF16, tag=f"fc3w{ki}")
                    nc.sync.dma_start(t_[:], fc3w_t[p0:p0 + kk, :])
                    f3w.append(t_)
                f3b = pool.tile([1, 256], BF16, tag="fc3b")
                nc.sync.dma_start(f3b[:], fc3b_t[:, :])
                w1 = []
                for ki, (p0, kk) in enumerate(((0, P), (P, 64))):
                    t_ = pool.tile([kk, P], BF16, tag=f"l1w{ki}")
                    nc.sync.dma_start(t_[:], l1w_t[p0:p0 + kk, :])
                    w1.append(t_)
                b1r = pool.tile([1, P], BF16, tag="l1b")
                nc.sync.dma_start(b1r[:], l1b_t[:, :])
                w2 = pool.tile([P, 64], BF16, tag="l2w")
                nc.sync.dma_start(w2[:], l2w_t[:, :])
                b2r = pool.tile([1, 64], BF16, tag="l2b")
                nc.sync.dma_start(b2r[:], l2b_t[:, :])
                wo = pool.tile([64, 8], BF16, tag="ow")
                nc.sync.dma_start(wo[:], ow_t[:, :])
                bo = pool.tile([1, 8], BF16, tag="ob")
                nc.sync.dma_start(bo[:], ob_t[:, :])
                CP = 512
                for s0, n in supertiles():
                    cT = gtr_piece(c3_d, 384, pool, "c3T", s0, n)
                    for c0 in range(s0, s0 + n, CP):
                        cw = min(CP, s0 + n - c0)
                        j0 = c0 - s0
                        msl = mask[0:1, c0:c0 + cw]
                        f3o = []
                        for ko, (o0, oc) in enumerate(((0, P), (P, 64))):
                            ps = psp.tile([oc, CP], F32, space="PSUM",
                                          tag=f"f3ps{ko}")
                            for ki, (c, kk) in enumerate(((0, P), (1, P),
                                                          (2, 32))):
                                nc.tensor.matmul(
                                    ps[:, :cw],
                                    lhsT=f3w[ki][:kk, o0:o0 + oc],
                                    rhs=cT[:kk, c, j0:j0 + cw],
                                    start=(ki == 0), stop=False)
                            nc.tensor.matmul(ps[:, :cw],
                                             lhsT=f3b[0:1, o0:o0 + oc],
                                             rhs=msl, start=False, stop=True)
                            ot = pool.tile([oc, CP], BF16, tag=f"f3o{ko}")
                            nc.scalar.activation(ot[:, :cw], ps[:, :cw],
                                                 ACT.Lrelu, alpha=SLOPE)
                            f3o.append(ot)
                        ps1 = psp.tile([P, CP], F32, space="PSUM", tag="l1ps")
                        for ki, kk in enumerate((P, 64)):
                            nc.tensor.matmul(ps1[:, :cw], lhsT=w1[ki][:],
                                             rhs=f3o[ki][:kk, :cw],
                                             start=(ki == 0), stop=False)
                        nc.tensor.matmul(ps1[:, :cw], lhsT=b1r[:],
                                         rhs=msl, start=False, stop=True)
                        l1o = pool.tile([P, CP], BF16, tag="l1o")
                        nc.scalar.activation(l1o[:, :cw], ps1[:, :cw],
                                             ACT.Copy)
                        ps2 = psp.tile([64, CP], F32, space="PSUM",
                                       tag="l2ps")
                        nc.tensor.matmul(ps2[:, :cw], lhsT=w2[:],
                                         rhs=l1o[:, :cw],
                                         start=True, stop=False)
                        nc.tensor.matmul(ps2[:, :cw], lhsT=b2r[:],
                                         rhs=msl, start=False, stop=True)
                        l2o = pool.tile([64, CP], BF16, tag="l2o")
                        nc.scalar.activation(l2o[:, :cw], ps2[:, :cw],
                                             ACT.Copy)
                        ps3 = psp.tile([8, CP], F32, space="PSUM", tag="ops")
                        nc.tensor.matmul(ps3[:, :cw], lhsT=wo[:],
                                         rhs=l2o[:, :cw],
                                         start=True, stop=False)
                        nc.tensor.matmul(ps3[:, :cw], lhsT=bo[:],
                                         rhs=msl, start=False, stop=True)
                        oo = pool.tile([8, CP], F32, tag="oo")
                        nc.scalar.activation(oo[:, :cw], ps3[:, :cw],
                                             ACT.Sigmoid)
                        nc.sync.dma_start(outT_t[:, c0:c0 + cw], oo[:, :cw])

    nc.compile()
    return nc


# ---------------------------------------------------------------------------
# Input packing
# ---------------------------------------------------------------------------

def _pack_inputs(plan, x, wd):
    R, M = plan.R, plan.M

    x_in = np.zeros((NCORES, R, P), BF)
    x_in[plan.core_of, plan.local, 0:3] = x.astype(BF)

    mask = np.zeros((NCORES, 1, R), BF)
    mask[plan.core_of, 0, plan.local] = 1.0

    iota = np.tile(np.arange(WIN, dtype=np.float32), (P, 1)).astype(BF)

    # weight blob (bf16): conv2 tiles then conv3 tiles
    blob = []
    w2l = _pad3(wd["Wl2"], NB, 192, 384).astype(BF)
    w2r = _pad3(wd["Wr2"], NB, 192, 384).astype(BF)
    for bkt in range(NB):
        for wmat in (w2l, w2r):
            for p0, kk in ((0, P), (P, 64)):
                blob.append(wmat[bkt, p0:p0 + kk, :].ravel())
    w3off = sum(b.size for b in blob)
    w3l = _pad3(wd["Wl3"], NB, 384, 384).astype(BF)
    w3r = _pad3(wd["Wr3"], NB, 384, 384).astype(BF)
    for bkt in range(NB):
        for wmat in (w3l, w3r):
            for ki in range(3):
                blob.append(wmat[bkt, P * ki:P * (ki + 1), :].ravel())
    blob = np.concatenate(blob)
    WSH = _ceil(len(blob), NCORES)
    blobp = np.zeros(NCORES * WSH, BF)
    blobp[: len(blob)] = blob
    plan.WSH = WSH
    plan.W3OFF = w3off

    def b16(a):
        return np.ascontiguousarray(a).astype(BF)

    common = {
        "iota": iota,
        "seq": plan.seq,
        "w1l": b16(_pad3(wd["Wl1"], NB, 4, P).transpose(1, 0, 2)
                   .reshape(4, NB * P)),
        "w1r": b16(_pad3(wd["Wr1"], NB, 4, P).transpose(1, 0, 2)
                   .reshape(4, NB * P)),
        "b1": b16(_pad2(wd["bl1"], NB, P).reshape(1, NB * P)),
        "fc1w": b16(_pad2(wd["fc1W"], P, 256)),
        "fc1b": b16(_pad2(wd["fc1b"][None, :], 1, 256)),
        "b2": b16(_pad2(wd["bl2"], NB, 384).reshape(1, NB * 384)),
        "fc2w": b16(_pad2(wd["fc2W"], 384, 384)),
        "fc2b": b16(_pad2(wd["fc2b"][None, :], 1, 384)),
        "b3": b16(_pad2(wd["bl3"], NB, 384).reshape(1, NB * 384)),
        "fc3w": b16(_pad2(wd["fc3W"], 384, 256)),
        "fc3b": b16(_pad2(wd["fc3b"][None, :], 1, 256)),
        "l1w": b16(_pad2(wd["lin1W"], 192, P)),
        "l1b": b16(_pad2(wd["lin1b"][None, :], 1, P)),
        "l2w": b16(_pad2(wd["lin2W"], P, 64)),
        "l2b": b16(_pad2(wd["lin2b"][None, :], 1, 64)),
        "ow": b16(_pad2(wd["outW"], 64, 8)),
        "ob": b16(_pad2(wd["outb"][None, :], 1, 8)),
    }
    in_maps = []
    for c in range(NCORES):
        m = dict(common)
        m["x_in"] = x_in[c]
        m["mask"] = mask[c]
        m["gidx"] = plan.gidx_w[c]
        m["dstv"] = plan.dst_w[c]
        m["wsh"] = blobp[c * WSH:(c + 1) * WSH][None, :]
        in_maps.append(m)
    return in_maps


# ---------------------------------------------------------------------------
# kernel entry (with per-edge-hash caching of plan + program + jit)
# ---------------------------------------------------------------------------

_CACHE = {}

WEIGHT_KEYS = ["Wl1", "Wr1", "bl1", "fc1W", "fc1b", "Wl2", "Wr2", "bl2",
               "fc2W", "fc2b", "Wl3", "Wr3", "bl3", "fc3W", "fc3b",
               "lin1W", "lin1b", "lin2W", "lin2b", "outW", "outb"]


def _make_runner(nc):
    """Cacheable jitted runner for nc (adapted from bass2jax PJRT path)."""
    import jax
    from jax.sharding import Mesh, PartitionSpec
    from jax.experimental.shard_map import shard_map
    from concourse import bass2jax

    bass2jax.install_neuronx_cc_hook()
    partition_name = (nc.partition_id_tensor.name
                      if nc.partition_id_tensor else None)
    in_names, out_names, out_avals, zero_shapes = [], [], [], []
    for alloc in nc.m.functions[0].allocations:
        if not isinstance(alloc, mybir.MemoryLocationSet):
            continue
        name = alloc.memorylocations[0].name
        if alloc.kind == "ExternalInput":
            if name != partition_name:
                in_names.append(name)
        elif alloc.kind == "ExternalOutput":
            out_names.append(name)
            shape = tuple(alloc.tensor_shape)
            dtype = mybir.dt.np(alloc.dtype)
            out_avals.append(jax.core.ShapedArray(shape, dtype))
            zero_shapes.append((shape, dtype))
    n_params = len(in_names)
    n_outs = len(out_avals)
    in_names_all = list(in_names) + out_names + (
        [partition_name] if partition_name else [])
    donate = tuple(range(n_params, n_params + n_outs))

    def _body(*args):
        operands = list(args)
        if partition_name is not None:
            operands.append(bass2jax.partition_id_tensor())
        outs = bass2jax._bass_exec_p.bind(
            *operands, out_avals=tuple(out_avals),
            in_names=tuple(in_names_all), out_names=tuple(out_names),
            lowering_input_output_aliases=(), sim_require_finite=True,
            sim_require_nnan=True, nc=nc)
        return tuple(outs)

    devices = jax.devices()[:NCORES]
    mesh = Mesh(np.asarray(devices), ("core",))
    in_specs = (PartitionSpec("core"),) * (n_params + n_outs)
    out_specs = (PartitionSpec("core"),) * len(out_names)
    sharded = jax.jit(
        shard_map(_body, mesh=mesh, in_specs=in_specs,
                  out_specs=out_specs, check_rep=False),
        donate_argnums=donate, keep_unused=True)

    def run(in_maps):
        concat_in = [
            np.concatenate([np.asarray(m[name]) for m in in_maps], axis=0)
            for name in in_names]
        concat_zeros = [
            np.zeros((NCORES * s[0], *s[1:]), d) for (s, d) in zero_shapes]
        outs = sharded(*concat_in, *concat_zeros)
        return [
            {name: np.asarray(outs[i]).reshape(NCORES, *out_avals[i].shape)[c]
             for i, name in enumerate(out_names)}
            for c in range(NCORES)]

    return run


def kernel(**inputs):
    x = np.ascontiguousarray(np.asarray(inputs["x"], dtype=np.float32))
    edge_index = np.ascontiguousarray(
        np.asarray(inputs["edge_index"], dtype=np.int64))
    N = x.shape[0]

    ekey = (hashlib.blake2b(edge_index.tobytes(), digest_size=16).hexdigest(),
            N)

    wd = {k: np.asarray(inputs[k], np.float32) for k in WEIGHT_KEYS}

    if ekey in _CACHE:
        plan, run = _CACHE[ekey]
        in_maps = _pack_inputs(plan, x, wd)
    else:
        plan = _preprocess(N, np.asarray(edge_index[0]),
                           np.asarray(edge_index[1]))
        in_maps = _pack_inputs(plan, x, wd)   # sets plan.WSH / plan.W3OFF
        nc = _build(plan)
        run = _make_runner(nc)
        _CACHE[ekey] = (plan, run)

    results = run(in_maps)
    kernel._last_results = None

    out = np.empty((N, 6), np.float32)
    for c in range(NCORES):
        oT = np.asarray(results[c]["outT"])   # [8, R]
        rows = plan.rows_old[c]
        valid = rows >= 0
        out[rows[valid]] = oT[:6, valid].T
    return out
